# revision 14
# baseline (speedup 1.0000x reference)
"""Trainium2 Bass kernel for nn_CausalGraphVAE (B=512, T=32, N=512, H=64,
L=128, HEADS=4, D=64) on 8 NeuronCores, batch/target-node sharded 64/core.

Layout strategy: everything on-device lives transposed ([feature, batch])
so per-feature biases are per-partition ACT bias columns. The two GRUs run
as 32 serial steps with input gates batched into PSUM ahead of time and
the hidden-path matmuls accumulating into the same PSUM slices. The GATv2
attention computes e[i, :, :] per target node i via one Prelu activation
(bias = xr_i column, slope 0.2) and a block-diagonal attention matmul; the
[4, 512] PSUM rows are packed three-per-tile at base partitions 0/32/64,
evacuated with one ACT copy, and flattened into an [i, (h, j)] SBUF tile
where the softmax is per-partition arithmetic. exp() everywhere is
sigmoid(x)/sigmoid(-x) so the whole kernel uses a single ACT table set
(sigmoid_and_others: Sigmoid, Tanh, Prelu, Copy) -- no 2.7us table swaps.
"""

import sys

sys.path.insert(0, "/opt/trn_rl_repo")

import numpy as np

B, T, N, H, L, HEADS, D = 512, 32, 512, 64, 128, 4, 64
L2 = L // 2
HD = HEADS * D  # 256
NC = 8
BC = B // NC  # 64 batch rows / target nodes per core

_PROG = None  # cached (nc, meta)


def _build():
    import concourse.bass as bass
    import concourse.tile as tile
    from concourse import bacc, mybir
    from concourse.masks import make_identity

    FP = mybir.dt.float32
    AF = mybir.ActivationFunctionType
    OP = mybir.AluOpType
    ts = bass.ts

    nc = bacc.Bacc("TRN2", target_bir_lowering=False, debug=False, num_devices=NC)

    def din(name, shape):
        return nc.dram_tensor(name, shape, FP, kind="ExternalInput").ap()

    def dout(name, shape):
        return nc.dram_tensor(name, shape, FP, kind="ExternalOutput").ap()

    # ---- DRAM I/O ----
    i_xt = din("xt", [N, T, BC])          # X.T sharded: [N, T, b]
    i_eps1 = din("eps1T", [L, BC])
    i_eps2 = din("eps2T", [L, BC])
    i_tcT = din("tcT", [T, BC])
    i_esn = din("es_now_sh", [BC, N])
    i_esl = din("es_lag_sh", [BC, N])
    i_prior = din("prior_sh", [BC, N])

    WSPECS = {
        "enc_WihT": [N, 192], "enc_WhhT": [H, 192],
        "enc_brz": [128, 1], "enc_bin": [64, 1], "enc_bhn": [64, 1],
        "muWT": [H, L], "mu_b": [L, 1], "lvWT": [H, L], "lv_b": [L, 1],
        "flow_sWT": [L2, L2], "flow_sb": [L2, 1],
        "flow_tWT": [L2, L2], "flow_tb": [L2, 1],
        "tg_WihT": [T, 192], "tg_brz": [128, 1], "tg_bin": [64, 1],
        "tg_bhn": [64, 1],
        "ltnWT": [L, N], "ltn_b_cols": [128, 4],
        "gatWlT": [N, HD], "gat_bl_cols": [128, 2],
        "gatWrT": [N, HD], "gat_br_cols": [128, 2],
        "attT_A": [128, 4], "attT_B": [128, 4],
        "xemb_b_cols": [128, 2],
        "glWT_A": [128, 2], "glWT_B": [128, 2],
        "glb_m": [1, 1], "glb_lv": [1, 1],
        "dec_WihT": [L, 192], "dec_WhhT": [H, 192],
        "dec_brz": [128, 1], "dec_bin": [64, 1], "dec_bhn": [64, 1],
        "fcWT_aug": [H + 1, N],
    }
    iw = {k: din(k, v) for k, v in WSPECS.items()}

    o_recon = dout("o_recon", [T, BC, N])
    o_muT = dout("o_muT", [L, BC])
    o_lvT = dout("o_lvT", [L, BC])
    o_adjn = dout("o_adjn", [BC, N])
    o_adjl = dout("o_adjl", [BC, N])
    o_meanT = dout("o_meanT", [1, BC])
    o_stdT = dout("o_stdT", [1, BC])

    TW = 8           # encoder gi window (timesteps)
    NWIN = T // TW   # 4

    with tile.TileContext(nc) as tc:
        with (
            tc.tile_pool(name="w", bufs=1) as wp,
            tc.tile_pool(name="st", bufs=1) as st,      # persistent state
            tc.tile_pool(name="r2", bufs=2) as r2,      # rotating sbuf
            tc.tile_pool(name="r3", bufs=3) as r3,
            tc.tile_pool(name="lrp", bufs=4) as lrp,    # Prelu outputs
            tc.tile_pool(name="dram", bufs=1, space="DRAM") as dp,
        ):
            # ---- weights into SBUF (partition-split where needed) ----
            w = {}
            for name, (p, f) in WSPECS.items():
                if p <= 128:
                    t = wp.tile([p, f], FP, tag=name)
                    nc.sync.dma_start(t[:], iw[name])
                    w[name] = t
                else:
                    parts = []
                    for k in range(p // 128):
                        t = wp.tile([128, f], FP, tag=f"{name}{k}")
                        nc.sync.dma_start(t[:], iw[name][ts(k, 128), :])
                        parts.append(t)
                    w[name] = parts

            ident = wp.tile([128, 128], FP, tag="ident")
            make_identity(nc, ident[:])

            # ---- adjacency outputs (independent; fills idle engines) ----
            esn = st.tile([BC, N], FP, tag="esn")
            nc.sync.dma_start(esn[:], i_esn)
            pri = st.tile([BC, N], FP, tag="pri")
            nc.sync.dma_start(pri[:], i_prior)
            esl = st.tile([BC, N], FP, tag="esl")
            nc.sync.dma_start(esl[:], i_esl)
            a1 = st.tile([BC, N], FP, tag="a1")
            nc.vector.tensor_tensor(a1[:], esn[:], pri[:], OP.add)
            adjn = st.tile([BC, N], FP, tag="adjn")
            nc.scalar.activation(adjn[:], a1[:], AF.Sigmoid)
            nc.sync.dma_start(o_adjn, adjn[:])
            adjl = st.tile([BC, N], FP, tag="adjl")
            nc.scalar.activation(adjl[:], esl[:], AF.Sigmoid)
            nc.sync.dma_start(o_adjl, adjl[:])

            # ---- encoder GRU ----
            hT = st.tile([H, BC], FP, tag="hT")
            nc.vector.memset(hT[:], 0.0)
            eWih = w["enc_WihT"]
            with (
                tc.tile_pool(name="pgirz", bufs=2, space="PSUM") as pgirz,
                tc.tile_pool(name="pgin", bufs=2, space="PSUM") as pgin,
                tc.tile_pool(name="pghn", bufs=2, space="PSUM") as pghn,
                tc.tile_pool(name="pm", bufs=2, space="PSUM") as pm,
            ):
                for wi in range(NWIN):
                    xw = []
                    for p in range(4):
                        xt_ = r2.tile([128, TW * BC], FP, tag=f"xw{p}")
                        nc.sync.dma_start(
                            xt_[:].rearrange("p (t b) -> p t b", t=TW),
                            i_xt[ts(p, 128), ts(wi, TW), :],
                        )
                        xw.append(xt_)
                    girz = pgirz.tile([128, TW * BC], FP, tag="girz")
                    gin = pgin.tile([64, TW * BC], FP, tag="gin")
                    for p in range(4):
                        nc.tensor.matmul(
                            girz[:], eWih[p][:, 0:128], xw[p][:],
                            start=(p == 0), stop=False,
                        )
                        nc.tensor.matmul(
                            gin[:], eWih[p][:, 128:192], xw[p][:],
                            start=(p == 0), stop=(p == 3),
                        )
                    for tl in range(TW):
                        sl = ts(tl, BC)
                        nc.tensor.matmul(
                            girz[:, sl], w["enc_WhhT"][:, 0:128], hT[:],
                            start=False, stop=(tl == TW - 1),
                        )
                        rz = r3.tile([128, BC], FP, tag="rz")
                        nc.scalar.activation(
                            rz[:], girz[:, sl], AF.Sigmoid, bias=w["enc_brz"][:]
                        )
                        ghn = pghn.tile([64, BC], FP, tag="ghn")
                        nc.tensor.matmul(
                            ghn[:], w["enc_WhhT"][:, 128:192], hT[:],
                            start=True, stop=True,
                        )
                        t1 = r3.tile([64, BC], FP, tag="t1")
                        nc.vector.scalar_tensor_tensor(
                            t1[:], ghn[:], w["enc_bhn"][:], rz[0:64, :],
                            op0=OP.add, op1=OP.mult,
                        )
                        t2 = r3.tile([64, BC], FP, tag="t2")
                        nc.vector.tensor_tensor(t2[:], t1[:], gin[:, sl], OP.add)
                        nsb = r3.tile([64, BC], FP, tag="nsb")
                        nc.scalar.activation(
                            nsb[:], t2[:], AF.Tanh, bias=w["enc_bin"][:]
                        )
                        # h' = n + z*(h-n): stage (h-n) at base 64 to pair
                        # with the z-gate rows of rz (input bases must match)
                        dd = r3.tile([128, BC], FP, tag="dd")
                        nc.vector.tensor_tensor(
                            dd[64:128, :], hT[:], nsb[:], OP.subtract
                        )
                        zd = r3.tile([64, BC], FP, tag="zd")
                        nc.vector.tensor_tensor(
                            zd[:], rz[64:128, :], dd[64:128, :], OP.mult
                        )
                        nc.vector.tensor_tensor(hT[:], nsb[:], zd[:], OP.add)

                # ---- mu / logvar / reparam / flow (local b-shard) ----
                pmu = pm.tile([L, BC], FP, tag="pm")
                nc.tensor.matmul(pmu[:], w["muWT"][:], hT[:], start=True, stop=True)
                muT = st.tile([L, BC], FP, tag="muT")
                nc.scalar.activation(muT[:], pmu[:], AF.Identity, bias=w["mu_b"][:])
                nc.sync.dma_start(o_muT, muT[:])
                plv = pm.tile([L, BC], FP, tag="pm")
                nc.tensor.matmul(plv[:], w["lvWT"][:], hT[:], start=True, stop=True)
                lvT = st.tile([L, BC], FP, tag="lvT")
                nc.scalar.activation(lvT[:], plv[:], AF.Identity, bias=w["lv_b"][:])
                nc.sync.dma_start(o_lvT, lvT[:])

                # expf = exp(0.5*lv) = sig(.5lv)/sig(-.5lv)
                ea = r2.tile([L, BC], FP, tag="ea")
                nc.scalar.activation(ea[:], lvT[:], AF.Sigmoid, scale=0.5)
                eb = r2.tile([L, BC], FP, tag="eb")
                nc.scalar.activation(eb[:], lvT[:], AF.Sigmoid, scale=-0.5)
                ebr = r2.tile([L, BC], FP, tag="ebr")
                nc.vector.reciprocal(ebr[:], eb[:])
                expf = st.tile([L, BC], FP, tag="expf")
                nc.vector.tensor_tensor(expf[:], ea[:], ebr[:], OP.mult)

                eps1 = r2.tile([L, BC], FP, tag="eps1")
                nc.sync.dma_start(eps1[:], i_eps1)
                eps2 = r2.tile([L, BC], FP, tag="eps2")
                nc.sync.dma_start(eps2[:], i_eps2)
                u1_ = r2.tile([L, BC], FP, tag="u1_")
                nc.vector.tensor_tensor(u1_[:], eps1[:], expf[:], OP.mult)
                ZT = st.tile([L, BC], FP, tag="ZT")
                nc.vector.tensor_tensor(ZT[:], muT[:], u1_[:], OP.add)
                u2_ = r2.tile([L, BC], FP, tag="u2_")
                nc.vector.tensor_tensor(u2_[:], eps2[:], expf[:], OP.mult)
                Z2T = st.tile([L, BC], FP, tag="Z2T")
                nc.vector.tensor_tensor(Z2T[:], muT[:], u2_[:], OP.add)

                # flow: z2' = sig(sW@z1+sb)*z2 + (tW@z1+tb)
                ZfT = st.tile([L, BC], FP, tag="ZfT")
                z1T = ZT[0:64, :]
                pfs = pm.tile([64, BC], FP, tag="pm")
                nc.tensor.matmul(pfs[:], w["flow_sWT"][:], z1T, start=True, stop=True)
                # z2-side tiles live at base partition 64 so tensor_tensor
                # inputs pair with [64:128] slices (input bases must match)
                sT = r2.tile([128, BC], FP, tag="sT")
                nc.scalar.activation(
                    sT[64:128, :], pfs[:], AF.Sigmoid, bias=w["flow_sb"][:]
                )
                pft = pm.tile([64, BC], FP, tag="pm")
                nc.tensor.matmul(pft[:], w["flow_tWT"][:], z1T, start=True, stop=True)
                tT = r2.tile([128, BC], FP, tag="tT")
                nc.scalar.activation(
                    tT[64:128, :], pft[:], AF.Identity, bias=w["flow_tb"][:]
                )
                v1 = r2.tile([128, BC], FP, tag="v1")
                nc.vector.tensor_tensor(
                    v1[64:128, :], sT[64:128, :], ZT[64:128, :], OP.mult
                )
                z2p = r2.tile([128, BC], FP, tag="z2p")
                nc.vector.tensor_tensor(
                    z2p[64:128, :], v1[64:128, :], tT[64:128, :], OP.add
                )
                # one-step GRU on time_context: z2'' = z2' + (1-zg)*n
                tcT = r2.tile([T, BC], FP, tag="tcT")
                nc.sync.dma_start(tcT[:], i_tcT)
                ptgrz = pm.tile([128, BC], FP, tag="pm")
                nc.tensor.matmul(
                    ptgrz[:], w["tg_WihT"][:, 0:128], tcT[:], start=True, stop=True
                )
                ptgn = pm.tile([64, BC], FP, tag="pm")
                nc.tensor.matmul(
                    ptgn[:], w["tg_WihT"][:, 128:192], tcT[:], start=True, stop=True
                )
                rzg = r2.tile([128, BC], FP, tag="rzg")
                nc.scalar.activation(
                    rzg[:], ptgrz[:], AF.Sigmoid, bias=w["tg_brz"][:]
                )
                w1 = r2.tile([64, BC], FP, tag="w1")
                nc.vector.scalar_tensor_tensor(
                    w1[:], rzg[0:64, :], w["tg_bhn"][:], ptgn[:],
                    op0=OP.mult, op1=OP.add,
                )
                ng = r2.tile([128, BC], FP, tag="ng")
                nc.scalar.activation(
                    ng[64:128, :], w1[:], AF.Tanh, bias=w["tg_bin"][:]
                )
                w2 = r2.tile([128, BC], FP, tag="w2")
                nc.vector.tensor_tensor(
                    w2[64:128, :], rzg[64:128, :], ng[64:128, :], OP.mult
                )
                w3 = r2.tile([128, BC], FP, tag="w3")
                nc.vector.tensor_tensor(
                    w3[64:128, :], ng[64:128, :], w2[64:128, :], OP.subtract
                )
                nc.scalar.activation(ZfT[0:64, :], z1T, AF.Copy)
                nc.vector.tensor_tensor(
                    ZfT[64:128, :], z2p[64:128, :], w3[64:128, :], OP.add
                )

            # ---- AllGather ZfT across the 8 cores ----
            cc_in = dp.tile([L, BC], FP)
            cc_out = dp.tile([NC, L, BC], FP)
            nc.sync.dma_start(cc_in[:], ZfT[:])
            nc.gpsimd.collective_compute(
                "AllGather",
                mybir.AluOpType.bypass,
                replica_groups=[list(range(NC))],
                ins=[cc_in[:].opt()],
                outs=[cc_out[:].opt()],
            )
            ZfF = st.tile([L, B], FP, tag="ZfF")
            for s in range(NC):
                nc.sync.dma_start(ZfF[:, ts(s, BC)], cc_out[s])

            # ---- decoder GRU (depends only on Z2T; overlaps GAT) ----
            with (
                tc.tile_pool(name="pdrz", bufs=1, space="PSUM") as pdrz,
                tc.tile_pool(name="pdn", bufs=1, space="PSUM") as pdn,
                tc.tile_pool(name="pfc", bufs=2, space="PSUM") as pfc,
            ):
                # constant input-gate n-half (rz halves re-accumulated per t)
                pgn = pdn.tile([64, BC], FP, tag="pdn")
                nc.tensor.matmul(
                    pgn[:], w["dec_WihT"][:, 128:192], Z2T[:], start=True, stop=True
                )
                ginD = st.tile([64, BC], FP, tag="ginD")
                nc.scalar.activation(ginD[:], pgn[:], AF.Copy)

                hdT = st.tile([H + 1, BC], FP, tag="hdT")
                nc.vector.memset(hdT[0:64, :], 0.0)
                nc.vector.memset(hdT[64:65, :], 1.0)
                for t in range(T):
                    drz = pdrz.tile([128, BC], FP, tag="pdrz")
                    nc.tensor.matmul(
                        drz[:], w["dec_WhhT"][:, 0:128], hdT[0:64, :],
                        start=True, stop=False,
                    )
                    nc.tensor.matmul(
                        drz[:], w["dec_WihT"][:, 0:128], Z2T[:],
                        start=False, stop=True,
                    )
                    rzd = r3.tile([128, BC], FP, tag="rzd")
                    nc.scalar.activation(
                        rzd[:], drz[:], AF.Sigmoid, bias=w["dec_brz"][:]
                    )
                    dn = pdn.tile([64, BC], FP, tag="pdn")
                    nc.tensor.matmul(
                        dn[:], w["dec_WhhT"][:, 128:192], hdT[0:64, :],
                        start=True, stop=True,
                    )
                    t1d = r3.tile([64, BC], FP, tag="t1d")
                    nc.vector.scalar_tensor_tensor(
                        t1d[:], dn[:], w["dec_bhn"][:], rzd[0:64, :],
                        op0=OP.add, op1=OP.mult,
                    )
                    t2d = r3.tile([64, BC], FP, tag="t2d")
                    nc.vector.tensor_tensor(t2d[:], t1d[:], ginD[:], OP.add)
                    nd = r3.tile([64, BC], FP, tag="nd")
                    nc.scalar.activation(
                        nd[:], t2d[:], AF.Tanh, bias=w["dec_bin"][:]
                    )
                    ddd = r3.tile([128, BC], FP, tag="ddd")
                    nc.vector.tensor_tensor(
                        ddd[64:128, :], hdT[0:64, :], nd[:], OP.subtract
                    )
                    zdd = r3.tile([64, BC], FP, tag="zdd")
                    nc.vector.tensor_tensor(
                        zdd[:], rzd[64:128, :], ddd[64:128, :], OP.mult
                    )
                    nc.vector.tensor_tensor(hdT[0:64, :], nd[:], zdd[:], OP.add)

                    fcp = pfc.tile([BC, N], FP, tag="pfc")
                    nc.tensor.matmul(
                        fcp[:], hdT[:], w["fcWT_aug"][:], start=True, stop=True
                    )
                    fcs = r3.tile([BC, N], FP, tag="fcs")
                    if t % 2 == 0:
                        nc.scalar.copy(fcs[:], fcp[:])
                    else:
                        nc.vector.tensor_copy(fcs[:], fcp[:])
                    nc.sync.dma_start(o_recon[t], fcs[:])

                # ---- Zn / xl / xr ----
                with (
                    tc.tile_pool(name="p512", bufs=2, space="PSUM") as p512,
                ):
                    ZnT, ZnTm = [], []
                    for m in range(4):
                        pz = p512.tile([128, B], FP, tag="p512")
                        nc.tensor.matmul(
                            pz[:], w["ltnWT"][:, ts(m, 128)], ZfF[:],
                            start=True, stop=True,
                        )
                        zt = st.tile([128, B], FP, tag=f"ZnT{m}")
                        nc.scalar.activation(
                            zt[:], pz[:], AF.Identity,
                            bias=w["ltn_b_cols"][:, m : m + 1],
                        )
                        ZnT.append(zt)
                        pzm = p512.tile([128, BC], FP, tag="p512")
                        nc.tensor.matmul(
                            pzm[:], w["ltnWT"][:, ts(m, 128)], ZfT[:],
                            start=True, stop=True,
                        )
                        ztm = st.tile([128, BC], FP, tag=f"ZnTm{m}")
                        nc.scalar.activation(
                            ztm[:], pzm[:], AF.Identity,
                            bias=w["ltn_b_cols"][:, m : m + 1],
                        )
                        ZnTm.append(ztm)
                    xlT, xrT = [], []
                    for hf in range(2):
                        px = p512.tile([128, B], FP, tag="p512")
                        for p in range(4):
                            nc.tensor.matmul(
                                px[:], w["gatWlT"][p][:, ts(hf, 128)], ZnT[p][:],
                                start=(p == 0), stop=(p == 3),
                            )
                        xt_ = st.tile([128, B], FP, tag=f"xlT{hf}")
                        nc.scalar.activation(
                            xt_[:], px[:], AF.Identity,
                            bias=w["gat_bl_cols"][:, hf : hf + 1],
                        )
                        xlT.append(xt_)
                        pxr = p512.tile([128, BC], FP, tag="p512")
                        for p in range(4):
                            nc.tensor.matmul(
                                pxr[:], w["gatWrT"][p][:, ts(hf, 128)], ZnTm[p][:],
                                start=(p == 0), stop=(p == 3),
                            )
                        xr_ = st.tile([128, BC], FP, tag=f"xrT{hf}")
                        nc.scalar.activation(
                            xr_[:], pxr[:], AF.Identity,
                            bias=w["gat_br_cols"][:, hf : hf + 1],
                        )
                        xrT.append(xr_)
                    xl = []
                    for jb in range(4):
                        pxl = p512.tile([128, HD], FP, tag="p512")
                        for p in range(4):
                            nc.tensor.matmul(
                                pxl[:], ZnT[p][:, ts(jb, 128)], w["gatWlT"][p][:],
                                start=(p == 0), stop=(p == 3),
                            )
                        xt_ = st.tile([128, HD], FP, tag=f"xl{jb}")
                        if jb % 2 == 0:
                            nc.scalar.copy(xt_[:], pxl[:])
                        else:
                            nc.vector.tensor_copy(xt_[:], pxl[:])
                        xl.append(xt_)

                # ---- GAT e-phase: e[i, h, j] for the core's 64 targets ----
                etile = st.tile([BC, HEADS * B], FP, tag="etile")
                with tc.tile_pool(name="pse", bufs=2, space="PSUM") as pse:
                    ngrp = (BC + 2) // 3
                    for g in range(ngrp):
                        iis = [g * 3 + k for k in range(3) if g * 3 + k < BC]
                        pe = pse.tile([68, B], FP, tag="pse")
                        for k, i in enumerate(iis):
                            lrs = []
                            for hf in range(2):
                                lr = lrp.tile([128, B], FP, tag=f"lr{hf}")
                                if i % 4 != 3:
                                    nc.scalar.activation(
                                        lr[:], xlT[hf][:], AF.Prelu,
                                        bias=xrT[hf][:, i : i + 1], alpha=0.2,
                                    )
                                else:
                                    nc.vector.tensor_scalar(
                                        lr[:], xlT[hf][:],
                                        xrT[hf][:, i : i + 1], None, op0=OP.add,
                                    )
                                    nc.vector.scalar_tensor_tensor(
                                        lr[:], lr[:], 0.2, lr[:],
                                        op0=OP.mult, op1=OP.max,
                                    )
                                lrs.append(lr)
                            out = pe[k * 32 : k * 32 + 4, :]
                            nc.tensor.matmul(
                                out, w["attT_A"][:], lrs[0][:],
                                start=True, stop=False,
                            )
                            nc.tensor.matmul(
                                out, w["attT_B"][:], lrs[1][:],
                                start=False, stop=True,
                            )
                        stag = r2.tile([68, B], FP, tag="stag")
                        nc.scalar.copy(stag[:], pe[:])
                        for k, i in enumerate(iis):
                            nc.sync.dma_start(
                                etile[i : i + 1, :].rearrange(
                                    "o (h j) -> o h j", h=HEADS
                                ),
                                stag[k * 32 : k * 32 + 4, :],
                            )

                # ---- softmax over j (per (i, h) row-segment) ----
                su1 = st.tile([BC, HEADS * B], FP, tag="su1")
                nc.scalar.activation(su1[:], etile[:], AF.Sigmoid)
                su2 = st.tile([BC, HEADS * B], FP, tag="su2")
                nc.scalar.activation(su2[:], etile[:], AF.Sigmoid, scale=-1.0)
                su2r = st.tile([BC, HEADS * B], FP, tag="su2r")
                nc.vector.reciprocal(su2r[:], su2[:])
                expE = st.tile([BC, HEADS * B], FP, tag="expE")
                nc.vector.tensor_tensor(expE[:], su1[:], su2r[:], OP.mult)
                sums = r2.tile([BC, HEADS], FP, tag="sums")
                nc.vector.tensor_reduce(
                    sums[:],
                    expE[:].rearrange("p (h j) -> p h j", h=HEADS),
                    axis=mybir.AxisListType.X,
                    op=OP.add,
                )
                rs = r2.tile([BC, HEADS], FP, tag="rs")
                nc.vector.reciprocal(rs[:], sums[:])
                alpha = st.tile([BC, HEADS * B], FP, tag="alpha")
                for h in range(HEADS):
                    nc.vector.tensor_scalar(
                        alpha[:, ts(h, B)], expE[:, ts(h, B)],
                        rs[:, h : h + 1], None, op0=OP.mult,
                    )

                # ---- alpha^T via PE transpose; X_emb; ml; mean/std ----
                with (
                    tc.tile_pool(name="psm", bufs=2, space="PSUM") as psm,
                    tc.tile_pool(name="pxe", bufs=1, space="PSUM") as pxe,
                ):
                    alphaT = st.tile([128, 16 * 64], FP, tag="alphaT")
                    for h in range(HEADS):
                        for jb in range(4):
                            ptr = psm.tile([128, 64], FP, tag="psm")
                            nc.tensor.transpose(
                                ptr[:],
                                alpha[:, h * B + jb * 128 : h * B + (jb + 1) * 128],
                                ident[0:BC, 0:BC],
                            )
                            dst = alphaT[:, ts(h * 4 + jb, 64)]
                            if (h * 4 + jb) % 2 == 0:
                                nc.scalar.copy(dst, ptr[:])
                            else:
                                nc.vector.tensor_copy(dst, ptr[:])
                    pxeA = pxe.tile([128, BC], FP, tag="pxeA")
                    pxeB = pxe.tile([128, BC], FP, tag="pxeB")
                    for h in range(HEADS):
                        tgt = pxeA if h < 2 else pxeB
                        out = tgt[(h % 2) * 64 : (h % 2) * 64 + 64, :]
                        for jb in range(4):
                            # out[d, i] = sum_j xl[j, d] * alphaT[j, i]
                            nc.tensor.matmul(
                                out,
                                xl[jb][:, ts(h, 64)],
                                alphaT[:, ts(h * 4 + jb, 64)],
                                start=(jb == 0), stop=(jb == 3),
                            )
                    xemb = []
                    for hf, pt in enumerate((pxeA, pxeB)):
                        xe = st.tile([128, BC], FP, tag=f"xemb{hf}")
                        nc.scalar.activation(
                            xe[:], pt[:], AF.Identity,
                            bias=w["xemb_b_cols"][:, hf : hf + 1],
                        )
                        xemb.append(xe)
                    # two M=1 matmuls: a 1-partition slice at base 1 is
                    # not addressable by compute engines
                    pml_m = psm.tile([1, BC], FP, tag="psm")
                    nc.tensor.matmul(
                        pml_m[:], w["glWT_A"][:, 0:1], xemb[0][:],
                        start=True, stop=False,
                    )
                    nc.tensor.matmul(
                        pml_m[:], w["glWT_B"][:, 0:1], xemb[1][:],
                        start=False, stop=True,
                    )
                    meanT = st.tile([1, BC], FP, tag="meanT")
                    nc.scalar.activation(
                        meanT[:], pml_m[:], AF.Identity, bias=w["glb_m"][:]
                    )
                    nc.sync.dma_start(o_meanT, meanT[:])
                    pml_s = psm.tile([1, BC], FP, tag="psm")
                    nc.tensor.matmul(
                        pml_s[:], w["glWT_A"][:, 1:2], xemb[0][:],
                        start=True, stop=False,
                    )
                    nc.tensor.matmul(
                        pml_s[:], w["glWT_B"][:, 1:2], xemb[1][:],
                        start=False, stop=True,
                    )
                    lvraw = st.tile([1, BC], FP, tag="lvraw")
                    nc.scalar.activation(
                        lvraw[:], pml_s[:], AF.Identity, bias=w["glb_lv"][:]
                    )
                    lvc = st.tile([1, BC], FP, tag="lvc")
                    nc.vector.tensor_scalar(
                        lvc[:], lvraw[:], 2.0, -5.0, op0=OP.min, op1=OP.max
                    )
                    sa1 = st.tile([1, BC], FP, tag="sa1")
                    nc.scalar.activation(sa1[:], lvc[:], AF.Sigmoid, scale=0.5)
                    sb1 = st.tile([1, BC], FP, tag="sb1")
                    nc.scalar.activation(sb1[:], lvc[:], AF.Sigmoid, scale=-0.5)
                    sbr = st.tile([1, BC], FP, tag="sbr")
                    nc.vector.reciprocal(sbr[:], sb1[:])
                    stdT = st.tile([1, BC], FP, tag="stdT")
                    nc.vector.tensor_tensor(stdT[:], sa1[:], sbr[:], OP.mult)
                    nc.sync.dma_start(o_stdT, stdT[:])

    nc.compile()
    return nc


def _get_prog():
    global _PROG
    if _PROG is None:
        _PROG = _build()
    return _PROG


def _host_prep(inputs):
    f32 = np.float32

    def c(a):
        return np.ascontiguousarray(a, dtype=f32)

    X = inputs["X"]
    XT = np.ascontiguousarray(np.transpose(X, (2, 1, 0)))  # [N, T, B]
    eps1T = c(inputs["eps1"].T)
    eps2T = c(inputs["eps2"].T)
    tcT = c(inputs["time_context"].T)

    att = np.asarray(inputs["gat_att"], f32)
    attT_A = np.zeros((128, 4), f32)
    attT_B = np.zeros((128, 4), f32)
    for h in range(2):
        attT_A[h * 64 : (h + 1) * 64, h] = att[h]
    for h in range(2, 4):
        attT_B[(h - 2) * 64 : (h - 1) * 64, h] = att[h]

    col = lambda v: np.asarray(v, f32).reshape(-1, 1)
    weights = {
        "enc_WihT": c(inputs["enc_Wih"].T),
        "enc_WhhT": c(inputs["enc_Whh"].T),
        "enc_brz": col(inputs["enc_bih"][0:128] + inputs["enc_bhh"][0:128]),
        "enc_bin": col(inputs["enc_bih"][128:192]),
        "enc_bhn": col(inputs["enc_bhh"][128:192]),
        "muWT": c(inputs["mu_W"].T), "mu_b": col(inputs["mu_b"]),
        "lvWT": c(inputs["lv_W"].T), "lv_b": col(inputs["lv_b"]),
        "flow_sWT": c(inputs["flow_sW"].T), "flow_sb": col(inputs["flow_sb"]),
        "flow_tWT": c(inputs["flow_tW"].T), "flow_tb": col(inputs["flow_tb"]),
        "tg_WihT": c(inputs["tg_Wih"].T),
        "tg_brz": col(inputs["tg_bih"][0:128] + inputs["tg_bhh"][0:128]),
        "tg_bin": col(inputs["tg_bih"][128:192]),
        "tg_bhn": col(inputs["tg_bhh"][128:192]),
        "ltnWT": c(inputs["ltn_W"].T),
        "ltn_b_cols": c(np.asarray(inputs["ltn_b"], f32).reshape(4, 128).T),
        "gatWlT": c(inputs["gat_Wl"].T),
        "gat_bl_cols": c(np.asarray(inputs["gat_bl"], f32).reshape(2, 128).T),
        "gatWrT": c(inputs["gat_Wr"].T),
        "gat_br_cols": c(np.asarray(inputs["gat_br"], f32).reshape(2, 128).T),
        "attT_A": attT_A, "attT_B": attT_B,
        "xemb_b_cols": c(
            (np.asarray(inputs["gat_bl"], f32) + np.asarray(inputs["gat_bias"], f32))
            .reshape(2, 128).T
        ),
        "glWT_A": c(inputs["gl_W"].T[0:128]),
        "glWT_B": c(inputs["gl_W"].T[128:256]),
        "glb_m": col(inputs["gl_b"][0:1]), "glb_lv": col(inputs["gl_b"][1:2]),
        "dec_WihT": c(inputs["dec_Wih"].T),
        "dec_WhhT": c(inputs["dec_Whh"].T),
        "dec_brz": col(inputs["dec_bih"][0:128] + inputs["dec_bhh"][0:128]),
        "dec_bin": col(inputs["dec_bih"][128:192]),
        "dec_bhn": col(inputs["dec_bhh"][128:192]),
        "fcWT_aug": np.concatenate(
            [c(inputs["fc_W"].T), np.asarray(inputs["fc_b"], f32)[None, :]], axis=0
        ),
    }

    esn = np.asarray(inputs["es_now"], f32)
    esl = np.asarray(inputs["es_lag"], f32)
    prior = np.asarray(inputs["prior_adj"], f32)

    in_maps = []
    for cidx in range(NC):
        sl = slice(cidx * BC, (cidx + 1) * BC)
        m = {
            "xt": np.ascontiguousarray(XT[:, :, sl]),
            "eps1T": np.ascontiguousarray(eps1T[:, sl]),
            "eps2T": np.ascontiguousarray(eps2T[:, sl]),
            "tcT": np.ascontiguousarray(tcT[:, sl]),
            "es_now_sh": np.ascontiguousarray(esn[sl]),
            "es_lag_sh": np.ascontiguousarray(esl[sl]),
            "prior_sh": np.ascontiguousarray(prior[sl]),
        }
        m.update(weights)
        in_maps.append(m)
    return in_maps


def run(inputs, trace=False):
    from concourse.bass_utils import run_bass_kernel_spmd

    nc = _get_prog()
    in_maps = _host_prep(inputs)
    res = run_bass_kernel_spmd(
        nc, in_maps, core_ids=list(range(NC)), trace=trace
    )
    rs = res.results

    recon = np.concatenate(
        [np.transpose(r["o_recon"], (1, 0, 2)) for r in rs], axis=0
    )
    mu = np.concatenate([r["o_muT"].T for r in rs], axis=0)
    logvar = np.concatenate([r["o_lvT"].T for r in rs], axis=0)
    adjn = np.concatenate([r["o_adjn"] for r in rs], axis=0)
    adjl = np.concatenate([r["o_adjl"] for r in rs], axis=0)
    idx = np.arange(N)
    adjn[idx, idx] = 1e-8
    adjl[idx, idx] = 1e-8
    mean = np.concatenate([r["o_meanT"][0] for r in rs])[:, None]
    std = np.concatenate([r["o_stdT"][0] for r in rs])[:, None]

    out = (
        recon.astype(np.float32),
        mu.astype(np.float32),
        logvar.astype(np.float32),
        adjn.astype(np.float32),
        adjl.astype(np.float32),
        mean.astype(np.float32),
        std.astype(np.float32),
    )
    if trace:
        return out, res.exec_time_ns
    return out


def kernel(**inputs):
    return run(inputs)


# revision 28
# speedup vs baseline: 4.3369x; 4.3369x over previous
"""Trainium2 Bass kernel for nn_CausalGraphVAE (B=512, T=32, N=512, H=64,
L=128, HEADS=4, D=64) on 8 NeuronCores, batch/target-node sharded 64/core.

Layout strategy: everything on-device lives transposed ([feature, batch])
so per-feature biases are per-partition ACT bias columns. The two GRUs run
as 32 serial steps with input gates batched into PSUM ahead of time and
the hidden-path matmuls accumulating into the same PSUM slices. The GATv2
attention computes e[i, :, :] per target node i via one Prelu activation
(bias = xr_i column, slope 0.2) and a block-diagonal attention matmul; the
[4, 512] PSUM rows are packed three-per-tile at base partitions 0/32/64,
evacuated with one ACT copy, and flattened into an [i, (h, j)] SBUF tile
where the softmax is per-partition arithmetic. exp() everywhere is
sigmoid(x)/sigmoid(-x) so the whole kernel uses a single ACT table set
(sigmoid_and_others: Sigmoid, Tanh, Prelu, Copy) -- no 2.7us table swaps.
"""

import sys

sys.path.insert(0, "/opt/trn_rl_repo")

import numpy as np

B, T, N, H, L, HEADS, D = 512, 32, 512, 64, 128, 4, 64
L2 = L // 2
HD = HEADS * D  # 256
NC = 8
BC = B // NC  # 64 batch rows / target nodes per core

_PROG = None  # cached (nc, meta)

# weights with p <= 128 partitions: packed column-wise into one [128, F] DMA
SMALL_W = [
    ("enc_WhhT", 64, 192), ("enc_brz", 128, 1), ("enc_bin", 64, 1),
    ("enc_bhn", 64, 1), ("muWT", 64, 128), ("mu_b", 128, 1),
    ("lvWT", 64, 128), ("lv_b", 128, 1),
    ("flow_sWT", 64, 64), ("flow_sb", 64, 1),
    ("flow_tWT", 64, 64), ("flow_tb", 64, 1),
    ("tg_WihT", 32, 192), ("tg_brz", 128, 1), ("tg_bin", 64, 1),
    ("tg_bhn", 64, 1),
    ("ltnWT", 128, 512), ("ltn_b_cols", 128, 4),
    ("gat_bl_cols", 128, 2), ("gat_br_cols", 128, 2),
    ("xemb_b_cols", 128, 2),
    ("glWT_A", 128, 2), ("glWT_B", 128, 2),
    ("glb_m", 1, 1), ("glb_lv", 1, 1),
    ("dec_WihT", 128, 192), ("dec_WhhT", 64, 192),
    ("dec_brz", 128, 1), ("dec_bin", 64, 1), ("dec_bhn", 64, 1),
    ("fcWT_aug", 65, 512),
]
SMALL_F = sum(f for _, _, f in SMALL_W)
# weights with 512 partition rows: 4 per-k-slice packs of [128, 704]
BIG_W = [("enc_WihT", 192), ("gatWlT", 256), ("gatWrT", 256)]
BIG_F = sum(f for _, f in BIG_W)
# bf16-declared pack (att matrices feed the bf16 e-matmuls)
RP_W = [("attT_A", 128, 4), ("attT_B", 128, 4)]
RP_F = sum(f for _, _, f in RP_W)


def _build():
    import concourse.bass as bass
    import concourse.tile as tile
    from concourse import bacc, mybir
    from concourse.masks import make_identity

    FP = mybir.dt.float32
    AF = mybir.ActivationFunctionType
    OP = mybir.AluOpType
    ts = bass.ts

    nc = bacc.Bacc("TRN2", target_bir_lowering=False, debug=False, num_devices=NC)
    F32R = mybir.dt.float32r

    def r32(ap):
        # fp32 matmuls cost 4 cycles/row; float32r with N>=256 costs 1
        return ap.bitcast(F32R)

    def din(name, shape):
        return nc.dram_tensor(name, shape, FP, kind="ExternalInput").ap()

    def dout(name, shape):
        return nc.dram_tensor(name, shape, FP, kind="ExternalOutput").ap()

    # ---- DRAM I/O ----
    i_xt = nc.dram_tensor(
        "xt", [N, T, BC], mybir.dt.float32r, kind="ExternalInput"
    ).ap()  # X.T sharded: [N, T, b]; f32r for the batched input-gate matmuls
    i_eps1 = din("eps1T", [L, BC])
    i_eps2 = din("eps2T", [L, BC])
    i_tcT = din("tcT", [T, BC])
    i_esn = din("es_now_sh", [BC, N])
    i_esl = din("es_lag_sh", [BC, N])
    i_prior = din("prior_sh", [BC, N])

    i_wpack = din("wpack", [128, SMALL_F])
    i_big = [
        nc.dram_tensor(f"bigpack{k}", [128, BIG_F], mybir.dt.float32r,
                       kind="ExternalInput").ap()
        for k in range(4)
    ]
    i_rpack = nc.dram_tensor("rpack", [128, RP_F], mybir.dt.bfloat16,
                             kind="ExternalInput").ap()

    o_recon = dout("o_recon", [T, BC, N])
    o_muT = dout("o_muT", [L, BC])
    o_lvT = dout("o_lvT", [L, BC])
    o_adjn = dout("o_adjn", [BC, N])
    o_adjl = dout("o_adjl", [BC, N])
    o_meanT = dout("o_meanT", [1, BC])
    o_stdT = dout("o_stdT", [1, BC])

    TW = 8           # encoder gi window (timesteps)
    NWIN = T // TW   # 4

    with tile.TileContext(nc) as tc:
        with (
            tc.tile_pool(name="w", bufs=1) as wp,
            tc.tile_pool(name="st", bufs=1) as st,      # persistent state
            tc.tile_pool(name="r2", bufs=2) as r2,      # rotating sbuf
            tc.tile_pool(name="r3", bufs=3) as r3,
            tc.tile_pool(name="lrp", bufs=4) as lrp,    # Prelu outputs
            tc.tile_pool(name="dram", bufs=1, space="DRAM") as dp,
        ):
            # ---- weights: 1 packed DMA for small, 4 for the 512-row ones
            wpack = wp.tile([128, SMALL_F], FP, tag="wpack")
            nc.sync.dma_start(wpack[:], i_wpack)
            bigs = []
            for k in range(4):
                bt = wp.tile([128, BIG_F], F32R, tag=f"big{k}")
                nc.sync.dma_start(bt[:], i_big[k])
                bigs.append(bt)
            rpk = wp.tile([128, RP_F], mybir.dt.bfloat16, tag="rpack")
            nc.sync.dma_start(rpk[:], i_rpack)
            w = {}
            off = 0
            for name, p, f in SMALL_W:
                w[name] = wpack[0:p, off : off + f]
                off += f
            off = 0
            for name, f in BIG_W:
                w[name] = [bigs[k][:, off : off + f] for k in range(4)]
                off += f
            off = 0
            for name, p, f in RP_W:
                w[name] = rpk[0:p, off : off + f]
                off += f

            ident = wp.tile([128, 128], FP, tag="ident")
            make_identity(nc, ident[:])
            ones1 = wp.tile([128, 1], FP, tag="ones1")
            nc.vector.memset(ones1[:], 1.0)

            # ---- adjacency outputs (independent; fills idle engines) ----
            esn = st.tile([BC, N], FP, tag="esn")
            nc.sync.dma_start(esn[:], i_esn)
            pri = st.tile([BC, N], FP, tag="pri")
            nc.sync.dma_start(pri[:], i_prior)
            esl = st.tile([BC, N], FP, tag="esl")
            nc.sync.dma_start(esl[:], i_esl)
            a1 = st.tile([BC, N], FP, tag="a1")
            nc.vector.tensor_tensor(a1[:], esn[:], pri[:], OP.add)
            adjn = st.tile([BC, N], FP, tag="adjn")
            nc.scalar.activation(adjn[:], a1[:], AF.Sigmoid)
            nc.sync.dma_start(o_adjn, adjn[:])
            adjl = st.tile([BC, N], FP, tag="adjl")
            nc.scalar.activation(adjl[:], esl[:], AF.Sigmoid)
            nc.sync.dma_start(o_adjl, adjl[:])

            # ---- encoder GRU ----
            hT = st.tile([H, BC], FP, tag="hT")
            nc.vector.memset(hT[:], 0.0)
            eWih = w["enc_WihT"]
            with (
                tc.tile_pool(name="pgirz", bufs=2, space="PSUM") as pgirz,
                tc.tile_pool(name="pgin", bufs=2, space="PSUM") as pgin,
                tc.tile_pool(name="pghn", bufs=2, space="PSUM") as pghn,
                tc.tile_pool(name="pm", bufs=2, space="PSUM") as pm,
            ):
                for wi in range(NWIN):
                    xw = []
                    for p in range(4):
                        xt_ = r2.tile([128, TW * BC], F32R, tag=f"xw{p}")
                        eng = nc.sync if wi < 2 else nc.scalar
                        eng.dma_start(
                            xt_[:].rearrange("p (t b) -> p t b", t=TW),
                            i_xt[ts(p, 128), ts(wi, TW), :],
                        )
                        xw.append(xt_)
                    girz = pgirz.tile([128, TW * BC], FP, tag="girz")
                    gin = pgin.tile([64, TW * BC], FP, tag="gin")
                    for p in range(4):
                        nc.tensor.matmul(
                            girz[:], eWih[p][:, 0:128], xw[p][:],
                            start=(p == 0), stop=False,
                        )
                        nc.tensor.matmul(
                            gin[:], eWih[p][:, 128:192], xw[p][:],
                            start=(p == 0), stop=(p == 3),
                        )
                    for tl in range(TW):
                        sl = ts(tl, BC)
                        nc.tensor.matmul(
                            girz[:, sl], w["enc_WhhT"][:, 0:128], hT[:],
                            start=False, stop=(tl == TW - 1),
                        )
                        # r and z sigmoids split so z lands at base 0; the
                        # z-branch (1-z, z*h) runs off the critical path:
                        # h' = n*(1-z) + z*h needs only 7 serial links.
                        rt = r3.tile([64, BC], FP, tag="rt")
                        nc.scalar.activation(
                            rt[:], girz[0:64, sl], AF.Sigmoid,
                            bias=w["enc_brz"][0:64, :],
                        )
                        zt = r3.tile([64, BC], FP, tag="zt")
                        nc.scalar.activation(
                            zt[:], girz[64:128, sl], AF.Sigmoid,
                            bias=w["enc_brz"][64:128, :],
                        )
                        omz = r3.tile([64, BC], FP, tag="omz")
                        nc.vector.tensor_scalar(
                            omz[:], zt[:], -1.0, 1.0, op0=OP.mult, op1=OP.add
                        )
                        zh = r3.tile([64, BC], FP, tag="zh")
                        nc.vector.tensor_tensor(zh[:], zt[:], hT[:], OP.mult)
                        ghn = pghn.tile([64, BC], FP, tag="ghn")
                        nc.tensor.matmul(
                            ghn[:], w["enc_WhhT"][:, 128:192], hT[:],
                            start=True, stop=True,
                        )
                        t1 = r3.tile([64, BC], FP, tag="t1")
                        nc.vector.scalar_tensor_tensor(
                            t1[:], ghn[:], w["enc_bhn"][:], rt[:],
                            op0=OP.add, op1=OP.mult,
                        )
                        t2 = r3.tile([64, BC], FP, tag="t2")
                        nc.vector.tensor_tensor(t2[:], t1[:], gin[:, sl], OP.add)
                        nsb = r3.tile([64, BC], FP, tag="nsb")
                        nc.scalar.activation(
                            nsb[:], t2[:], AF.Tanh, bias=w["enc_bin"][:]
                        )
                        nn1 = r3.tile([64, BC], FP, tag="nn1")
                        nc.vector.tensor_tensor(nn1[:], nsb[:], omz[:], OP.mult)
                        nc.vector.tensor_tensor(hT[:], nn1[:], zh[:], OP.add)

                # ---- mu / logvar / reparam / flow (local b-shard) ----
                pmu = pm.tile([L, BC], FP, tag="pm")
                nc.tensor.matmul(pmu[:], w["muWT"][:], hT[:], start=True, stop=True)
                muT = st.tile([L, BC], FP, tag="muT")
                nc.scalar.activation(muT[:], pmu[:], AF.Identity, bias=w["mu_b"][:])
                nc.sync.dma_start(o_muT, muT[:])
                plv = pm.tile([L, BC], FP, tag="pm")
                nc.tensor.matmul(plv[:], w["lvWT"][:], hT[:], start=True, stop=True)
                lvT = st.tile([L, BC], FP, tag="lvT")
                nc.scalar.activation(lvT[:], plv[:], AF.Identity, bias=w["lv_b"][:])
                nc.sync.dma_start(o_lvT, lvT[:])

                # expf = exp(0.5*lv) = sig(.5lv)/sig(-.5lv)
                ea = r2.tile([L, BC], FP, tag="ea")
                nc.scalar.activation(ea[:], lvT[:], AF.Sigmoid, scale=0.5)
                eb = r2.tile([L, BC], FP, tag="eb")
                nc.scalar.activation(eb[:], lvT[:], AF.Sigmoid, scale=-0.5)
                ebr = r2.tile([L, BC], FP, tag="ebr")
                nc.vector.reciprocal(ebr[:], eb[:])
                expf = st.tile([L, BC], FP, tag="expf")
                nc.vector.tensor_tensor(expf[:], ea[:], ebr[:], OP.mult)

                eps1 = r2.tile([L, BC], FP, tag="eps1")
                nc.sync.dma_start(eps1[:], i_eps1)
                eps2 = r2.tile([L, BC], FP, tag="eps2")
                nc.sync.dma_start(eps2[:], i_eps2)
                u1_ = r2.tile([L, BC], FP, tag="u1_")
                nc.vector.tensor_tensor(u1_[:], eps1[:], expf[:], OP.mult)
                ZT = st.tile([L, BC], FP, tag="ZT")
                nc.vector.tensor_tensor(ZT[:], muT[:], u1_[:], OP.add)
                u2_ = r2.tile([L, BC], FP, tag="u2_")
                nc.vector.tensor_tensor(u2_[:], eps2[:], expf[:], OP.mult)
                Z2T = st.tile([L, BC], FP, tag="Z2T")
                nc.vector.tensor_tensor(Z2T[:], muT[:], u2_[:], OP.add)

                # flow: z2' = sig(sW@z1+sb)*z2 + (tW@z1+tb)
                ZfT = st.tile([L, BC], FP, tag="ZfT")
                z1T = ZT[0:64, :]
                pfs = pm.tile([64, BC], FP, tag="pm")
                nc.tensor.matmul(pfs[:], w["flow_sWT"][:], z1T, start=True, stop=True)
                # z2-side tiles live at base partition 64 so tensor_tensor
                # inputs pair with [64:128] slices (input bases must match)
                sT = r2.tile([128, BC], FP, tag="sT")
                nc.scalar.activation(
                    sT[64:128, :], pfs[:], AF.Sigmoid, bias=w["flow_sb"][:]
                )
                pft = pm.tile([64, BC], FP, tag="pm")
                nc.tensor.matmul(pft[:], w["flow_tWT"][:], z1T, start=True, stop=True)
                tT = r2.tile([128, BC], FP, tag="tT")
                nc.scalar.activation(
                    tT[64:128, :], pft[:], AF.Identity, bias=w["flow_tb"][:]
                )
                v1 = r2.tile([128, BC], FP, tag="v1")
                nc.vector.tensor_tensor(
                    v1[64:128, :], sT[64:128, :], ZT[64:128, :], OP.mult
                )
                z2p = r2.tile([128, BC], FP, tag="z2p")
                nc.vector.tensor_tensor(
                    z2p[64:128, :], v1[64:128, :], tT[64:128, :], OP.add
                )
                # one-step GRU on time_context: z2'' = z2' + (1-zg)*n
                tcT = r2.tile([T, BC], FP, tag="tcT")
                nc.sync.dma_start(tcT[:], i_tcT)
                ptgrz = pm.tile([128, BC], FP, tag="pm")
                nc.tensor.matmul(
                    ptgrz[:], w["tg_WihT"][:, 0:128], tcT[:], start=True, stop=True
                )
                ptgn = pm.tile([64, BC], FP, tag="pm")
                nc.tensor.matmul(
                    ptgn[:], w["tg_WihT"][:, 128:192], tcT[:], start=True, stop=True
                )
                rzg = r2.tile([128, BC], FP, tag="rzg")
                nc.scalar.activation(
                    rzg[:], ptgrz[:], AF.Sigmoid, bias=w["tg_brz"][:]
                )
                w1 = r2.tile([64, BC], FP, tag="w1")
                nc.vector.scalar_tensor_tensor(
                    w1[:], rzg[0:64, :], w["tg_bhn"][:], ptgn[:],
                    op0=OP.mult, op1=OP.add,
                )
                ng = r2.tile([128, BC], FP, tag="ng")
                nc.scalar.activation(
                    ng[64:128, :], w1[:], AF.Tanh, bias=w["tg_bin"][:]
                )
                w2 = r2.tile([128, BC], FP, tag="w2")
                nc.vector.tensor_tensor(
                    w2[64:128, :], rzg[64:128, :], ng[64:128, :], OP.mult
                )
                w3 = r2.tile([128, BC], FP, tag="w3")
                nc.vector.tensor_tensor(
                    w3[64:128, :], ng[64:128, :], w2[64:128, :], OP.subtract
                )
                nc.scalar.activation(ZfT[0:64, :], z1T, AF.Copy)
                nc.vector.tensor_tensor(
                    ZfT[64:128, :], z2p[64:128, :], w3[64:128, :], OP.add
                )

            # ---- AllGather ZfT across the 8 cores ----
            cc_in = dp.tile([L, BC], FP)
            cc_out = dp.tile([NC, L, BC], FP)
            nc.sync.dma_start(cc_in[:], ZfT[:])
            nc.gpsimd.collective_compute(
                "AllGather",
                mybir.AluOpType.bypass,
                replica_groups=[list(range(NC))],
                ins=[cc_in[:].opt()],
                outs=[cc_out[:].opt()],
            )
            ZfF = st.tile([L, B], FP, tag="ZfF")
            for s in range(NC):
                nc.sync.dma_start(ZfF[:, ts(s, BC)], cc_out[s])

            # ---- decoder GRU (depends only on Z2T; overlaps GAT) ----
            with (
                tc.tile_pool(name="pdrz", bufs=1, space="PSUM") as pdrz,
                tc.tile_pool(name="pdn", bufs=1, space="PSUM") as pdn,
                tc.tile_pool(name="pfc", bufs=2, space="PSUM") as pfc,
            ):
                # constant input-gate n-half (rz halves re-accumulated per t)
                pgn = pdn.tile([64, BC], FP, tag="pdn")
                nc.tensor.matmul(
                    pgn[:], w["dec_WihT"][:, 128:192], Z2T[:], start=True, stop=True
                )
                ginD = st.tile([64, BC], FP, tag="ginD")
                nc.scalar.activation(ginD[:], pgn[:], AF.Copy)

                hdT = st.tile([H + 1, BC], FP, tag="hdT")
                nc.vector.memset(hdT[0:64, :], 0.0)
                nc.vector.memset(hdT[64:65, :], 1.0)
                for t in range(T):
                    drz = pdrz.tile([128, BC], FP, tag="pdrz")
                    nc.tensor.matmul(
                        drz[:], w["dec_WhhT"][:, 0:128], hdT[0:64, :],
                        start=True, stop=False,
                    )
                    nc.tensor.matmul(
                        drz[:], w["dec_WihT"][:, 0:128], Z2T[:],
                        start=False, stop=True,
                    )
                    rzd = r3.tile([128, BC], FP, tag="rzd")
                    nc.scalar.activation(
                        rzd[:], drz[:], AF.Sigmoid, bias=w["dec_brz"][:]
                    )
                    dn = pdn.tile([64, BC], FP, tag="pdn")
                    nc.tensor.matmul(
                        dn[:], w["dec_WhhT"][:, 128:192], hdT[0:64, :],
                        start=True, stop=True,
                    )
                    t1d = r3.tile([64, BC], FP, tag="t1d")
                    nc.vector.scalar_tensor_tensor(
                        t1d[:], dn[:], w["dec_bhn"][:], rzd[0:64, :],
                        op0=OP.add, op1=OP.mult,
                    )
                    t2d = r3.tile([64, BC], FP, tag="t2d")
                    nc.vector.tensor_tensor(t2d[:], t1d[:], ginD[:], OP.add)
                    nd = r3.tile([64, BC], FP, tag="nd")
                    nc.scalar.activation(
                        nd[:], t2d[:], AF.Tanh, bias=w["dec_bin"][:]
                    )
                    ddd = r3.tile([128, BC], FP, tag="ddd")
                    nc.vector.tensor_tensor(
                        ddd[64:128, :], hdT[0:64, :], nd[:], OP.subtract
                    )
                    zdd = r3.tile([64, BC], FP, tag="zdd")
                    nc.vector.tensor_tensor(
                        zdd[:], rzd[64:128, :], ddd[64:128, :], OP.mult
                    )
                    nc.vector.tensor_tensor(hdT[0:64, :], nd[:], zdd[:], OP.add)

                    fcp = pfc.tile([BC, N], FP, tag="pfc")
                    nc.tensor.matmul(
                        fcp[:], hdT[:], w["fcWT_aug"][:], start=True, stop=True
                    )
                    fcs = r3.tile([BC, N], FP, tag="fcs")
                    if t % 2 == 0:
                        nc.scalar.copy(fcs[:], fcp[:])
                    else:
                        nc.vector.tensor_copy(fcs[:], fcp[:])
                    nc.gpsimd.dma_start(o_recon[t], fcs[:])

                # ---- Zn / xl / xr ----
                with (
                    tc.tile_pool(name="p512", bufs=2, space="PSUM") as p512,
                ):
                    ZnT, ZnTm = [], []
                    for m in range(4):
                        pz = p512.tile([128, B], FP, tag="p512")
                        nc.tensor.matmul(
                            pz[:], w["ltnWT"][:, ts(m, 128)], ZfF[:],
                            start=True, stop=True,
                        )
                        zt = st.tile([128, B], F32R, tag=f"ZnT{m}")
                        nc.scalar.activation(
                            zt[:], pz[:], AF.Identity,
                            bias=w["ltn_b_cols"][:, m : m + 1],
                        )
                        ZnT.append(zt)
                        pzm = p512.tile([128, BC], FP, tag="p512")
                        nc.tensor.matmul(
                            pzm[:], w["ltnWT"][:, ts(m, 128)], ZfT[:],
                            start=True, stop=True,
                        )
                        ztm = st.tile([128, BC], F32R, tag=f"ZnTm{m}")
                        nc.scalar.activation(
                            ztm[:], pzm[:], AF.Identity,
                            bias=w["ltn_b_cols"][:, m : m + 1],
                        )
                        ZnTm.append(ztm)
                    xlT, xrT = [], []
                    for hf in range(2):
                        px = p512.tile([128, B], FP, tag="p512")
                        for p in range(4):
                            nc.tensor.matmul(
                                px[:], w["gatWlT"][p][:, ts(hf, 128)],
                                ZnT[p][:],
                                start=(p == 0), stop=(p == 3),
                            )
                        xt_ = st.tile([128, B], mybir.dt.bfloat16, tag=f"xlT{hf}")
                        nc.scalar.activation(
                            xt_[:], px[:], AF.Identity,
                            bias=w["gat_bl_cols"][:, hf : hf + 1],
                        )
                        xlT.append(xt_)
                        pxr = p512.tile([128, BC], FP, tag="p512")
                        for p in range(4):
                            nc.tensor.matmul(
                                pxr[:], w["gatWrT"][p][:, ts(hf, 128)], ZnTm[p][:],
                                start=(p == 0), stop=(p == 3),
                            )
                        xr_ = st.tile([128, BC], FP, tag=f"xrT{hf}")
                        nc.scalar.activation(
                            xr_[:], pxr[:], AF.Identity,
                            bias=w["gat_br_cols"][:, hf : hf + 1],
                        )
                        xrT.append(xr_)
                    xl = []
                    for jb in range(4):
                        pxl = p512.tile([128, HD], FP, tag="p512")
                        for p in range(4):
                            nc.tensor.matmul(
                                pxl[:], ZnT[p][:, ts(jb, 128)],
                                w["gatWlT"][p][:],
                                start=(p == 0), stop=(p == 3),
                            )
                        xt_ = st.tile([128, HD], FP, tag=f"xl{jb}")
                        if jb % 2 == 0:
                            nc.scalar.copy(xt_[:], pxl[:])
                        else:
                            nc.vector.tensor_copy(xt_[:], pxl[:])
                        xl.append(xt_)

                # ---- GAT e-phase ----
                # For each target i: lr = Prelu(xlT + xr_i, 0.2); e rows
                # [4, 512] land at psum base 32k (3 targets/tile). The staged
                # [68, 512] block is PE-transposed into ej tiles with layout
                # [j-part, (group, islot*32 + h)] so softmax sums become
                # ones-matmuls and normalization happens after X_emb.
                NG = (BC + 2) // 3  # 22
                ej = [
                    st.tile([128, NG * 96], FP, tag=f"ej{jb}", name=f"ej{jb}")
                    for jb in range(4)
                ]
                with (
                    tc.tile_pool(name="pse", bufs=2, space="PSUM") as pse,
                    tc.tile_pool(name="ptr", bufs=2, space="PSUM") as ptrp,
                ):
                    ngrp = (BC + 2) // 3
                    for g in range(ngrp):
                        iis = [g * 3 + k for k in range(3) if g * 3 + k < BC]
                        pe = pse.tile([68, B], FP, tag="pse")
                        for k, i in enumerate(iis):
                            lrs = []
                            for hf in range(2):
                                lr = lrp.tile(
                                    [128, B], mybir.dt.bfloat16, tag=f"lr{hf}"
                                )
                                # split the broadcast-add+leakyrelu across
                                # ACT / DVE / (gpsimd-add + DVE-max)
                                if i % 2 == 0:
                                    nc.scalar.activation(
                                        lr[:], xlT[hf][:], AF.Prelu,
                                        bias=xrT[hf][:, i : i + 1], alpha=0.2,
                                    )
                                elif i % 4 == 1:
                                    nc.vector.tensor_scalar(
                                        lr[:], xlT[hf][:],
                                        xrT[hf][:, i : i + 1], None, op0=OP.add,
                                    )
                                    nc.vector.scalar_tensor_tensor(
                                        lr[:], lr[:], 0.2, lr[:],
                                        op0=OP.mult, op1=OP.max,
                                    )
                                else:
                                    nc.gpsimd.tensor_scalar(
                                        lr[:], xlT[hf][:],
                                        xrT[hf][:, i : i + 1], None, op0=OP.add,
                                    )
                                    nc.vector.scalar_tensor_tensor(
                                        lr[:], lr[:], 0.2, lr[:],
                                        op0=OP.mult, op1=OP.max,
                                    )
                                lrs.append(lr)
                            out = pe[k * 32 : k * 32 + 4, :]
                            nc.tensor.matmul(
                                out, w["attT_A"][:], lrs[0][:],
                                start=True, stop=False,
                            )
                            nc.tensor.matmul(
                                out, w["attT_B"][:], lrs[1][:],
                                start=False, stop=True,
                            )
                        stag = r2.tile([68, B], FP, tag="stag")
                        if g % 2 == 0:
                            nc.scalar.copy(stag[:], pe[:])
                        else:
                            nc.vector.tensor_copy(stag[:], pe[:])
                        for jb in range(4):
                            ptr = ptrp.tile([128, 68], FP, tag="ptr")
                            nc.tensor.transpose(
                                ptr[:], stag[:, ts(jb, 128)], ident[0:68, 0:68]
                            )
                            dst = ej[jb][:, g * 96 : g * 96 + 68]
                            if (g * 4 + jb) % 2 == 0:
                                nc.scalar.copy(dst, ptr[:])
                            else:
                                nc.vector.tensor_copy(dst, ptr[:])

                # ---- exp(e) on the used (i,h) columns; sums via ones-mm
                def ev(t, hsl=None):
                    v = t[:].rearrange("p (g s r) -> p g s r", s=3, r=32)
                    return v[:, :, :, 0:4] if hsl is None else v[:, :, :, hsl]

                u1 = st.tile([128, NG * 96], FP, tag="u1s")
                u2 = st.tile([128, NG * 96], FP, tag="u2s")
                for jb in range(4):
                    nc.scalar.activation(ev(u1), ev(ej[jb]), AF.Sigmoid)
                    nc.scalar.activation(ev(u2), ev(ej[jb]), AF.Sigmoid, scale=-1.0)
                    nc.vector.reciprocal(ev(u2), ev(u2))
                    nc.vector.tensor_tensor(
                        ev(ej[jb]), ev(u1), ev(u2), OP.mult
                    )

                with (
                    tc.tile_pool(name="pxs", bufs=1, space="PSUM") as pxs,
                    tc.tile_pool(name="pxe", bufs=1, space="PSUM") as pxe,
                    tc.tile_pool(name="psm", bufs=1, space="PSUM") as psm,
                ):
                    psums = pxs.tile([1, 264], FP, tag="psums")
                    for jb in range(4):
                        nc.tensor.matmul(
                            psums[:], ones1[:], ev(ej[jb]),
                            start=(jb == 0), stop=(jb == 3),
                        )
                    recs = st.tile([1, 264], FP, tag="recs")
                    nc.vector.reciprocal(recs[:], psums[:])
                    # per-head reciprocal rows broadcast down 64 d-partitions
                    dvp = []
                    for h in range(HEADS):
                        d_ = st.tile([64, 64], FP, tag=f"dv{h}")
                        rv = recs[:].rearrange("o (i r) -> o i r", r=4)
                        nc.gpsimd.partition_broadcast(
                            d_[:], rv[:, 0:64, h : h + 1], channels=64
                        )
                        dvp.append(d_)
                    dva = st.tile([128, 64], FP, tag="dva")
                    nc.scalar.copy(dva[0:64, :], dvp[0][:])
                    nc.scalar.copy(dva[64:128, :], dvp[1][:])
                    dvb = st.tile([128, 64], FP, tag="dvb")
                    nc.scalar.copy(dvb[0:64, :], dvp[2][:])
                    nc.scalar.copy(dvb[64:128, :], dvp[3][:])

                    pxeA = pxe.tile([128, 66], FP, tag="pxeA")
                    pxeB = pxe.tile([128, 66], FP, tag="pxeB")
                    for h in range(HEADS):
                        tgt = pxeA if h < 2 else pxeB
                        out = tgt[(h % 2) * 64 : (h % 2) * 64 + 64, :]
                        for jb in range(4):
                            # out[d, i] = sum_j xl[j, d] * expE[j, (i, h)]
                            nc.tensor.matmul(
                                out,
                                xl[jb][:, ts(h, 64)],
                                ev(ej[jb], slice(h, h + 1)),
                                start=(jb == 0), stop=(jb == 3),
                            )
                    xemb = []
                    for hf, (pt, dv_) in enumerate(((pxeA, dva), (pxeB, dvb))):
                        xn = st.tile([128, BC], FP, tag=f"xn{hf}")
                        nc.vector.tensor_tensor(
                            xn[:], pt[:, 0:64], dv_[:], OP.mult
                        )
                        xe = st.tile([128, BC], FP, tag=f"xemb{hf}")
                        nc.scalar.activation(
                            xe[:], xn[:], AF.Identity,
                            bias=w["xemb_b_cols"][:, hf : hf + 1],
                        )
                        xemb.append(xe)
                    # two M=1 matmuls for the 2-row gl head
                    pml_m = psm.tile([1, BC], FP, tag="psm")
                    nc.tensor.matmul(
                        pml_m[:], w["glWT_A"][:, 0:1], xemb[0][:],
                        start=True, stop=False,
                    )
                    nc.tensor.matmul(
                        pml_m[:], w["glWT_B"][:, 0:1], xemb[1][:],
                        start=False, stop=True,
                    )
                    meanT = st.tile([1, BC], FP, tag="meanT")
                    nc.scalar.activation(
                        meanT[:], pml_m[:], AF.Identity, bias=w["glb_m"][:]
                    )
                    nc.sync.dma_start(o_meanT, meanT[:])
                    pml_s = psm.tile([1, BC], FP, tag="psm")
                    nc.tensor.matmul(
                        pml_s[:], w["glWT_A"][:, 1:2], xemb[0][:],
                        start=True, stop=False,
                    )
                    nc.tensor.matmul(
                        pml_s[:], w["glWT_B"][:, 1:2], xemb[1][:],
                        start=False, stop=True,
                    )
                    lvraw = st.tile([1, BC], FP, tag="lvraw")
                    nc.scalar.activation(
                        lvraw[:], pml_s[:], AF.Identity, bias=w["glb_lv"][:]
                    )
                    lvc = st.tile([1, BC], FP, tag="lvc")
                    nc.vector.tensor_scalar(
                        lvc[:], lvraw[:], 2.0, -5.0, op0=OP.min, op1=OP.max
                    )
                    sa1 = st.tile([1, BC], FP, tag="sa1")
                    nc.scalar.activation(sa1[:], lvc[:], AF.Sigmoid, scale=0.5)
                    sb1 = st.tile([1, BC], FP, tag="sb1")
                    nc.scalar.activation(sb1[:], lvc[:], AF.Sigmoid, scale=-0.5)
                    sbr = st.tile([1, BC], FP, tag="sbr")
                    nc.vector.reciprocal(sbr[:], sb1[:])
                    stdT = st.tile([1, BC], FP, tag="stdT")
                    nc.vector.tensor_tensor(stdT[:], sa1[:], sbr[:], OP.mult)
                    nc.sync.dma_start(o_stdT, stdT[:])

    nc.compile()
    return nc


def _get_prog():
    global _PROG
    if _PROG is None:
        _PROG = _build()
    return _PROG


def _host_prep(inputs):
    f32 = np.float32

    def c(a):
        return np.ascontiguousarray(a, dtype=f32)

    X = inputs["X"]
    XT = np.ascontiguousarray(np.transpose(X, (2, 1, 0)))  # [N, T, B]
    eps1T = c(inputs["eps1"].T)
    eps2T = c(inputs["eps2"].T)
    tcT = c(inputs["time_context"].T)

    att = np.asarray(inputs["gat_att"], f32)
    attT_A = np.zeros((128, 4), f32)
    attT_B = np.zeros((128, 4), f32)
    for h in range(2):
        attT_A[h * 64 : (h + 1) * 64, h] = att[h]
    for h in range(2, 4):
        attT_B[(h - 2) * 64 : (h - 1) * 64, h] = att[h]

    col = lambda v: np.asarray(v, f32).reshape(-1, 1)
    wvals = {
        "enc_WihT": c(inputs["enc_Wih"].T),  # [512, 192]
        "enc_WhhT": c(inputs["enc_Whh"].T),
        "enc_brz": col(inputs["enc_bih"][0:128] + inputs["enc_bhh"][0:128]),
        "enc_bin": col(inputs["enc_bih"][128:192]),
        "enc_bhn": col(inputs["enc_bhh"][128:192]),
        "muWT": c(inputs["mu_W"].T), "mu_b": col(inputs["mu_b"]),
        "lvWT": c(inputs["lv_W"].T), "lv_b": col(inputs["lv_b"]),
        "flow_sWT": c(inputs["flow_sW"].T), "flow_sb": col(inputs["flow_sb"]),
        "flow_tWT": c(inputs["flow_tW"].T), "flow_tb": col(inputs["flow_tb"]),
        "tg_WihT": c(inputs["tg_Wih"].T),
        "tg_brz": col(inputs["tg_bih"][0:128] + inputs["tg_bhh"][0:128]),
        "tg_bin": col(inputs["tg_bih"][128:192]),
        "tg_bhn": col(inputs["tg_bhh"][128:192]),
        "ltnWT": c(inputs["ltn_W"].T),
        "ltn_b_cols": c(np.asarray(inputs["ltn_b"], f32).reshape(4, 128).T),
        "gatWlT": c(inputs["gat_Wl"].T),
        "gat_bl_cols": c(np.asarray(inputs["gat_bl"], f32).reshape(2, 128).T),
        "gatWrT": c(inputs["gat_Wr"].T),
        "gat_br_cols": c(np.asarray(inputs["gat_br"], f32).reshape(2, 128).T),
        "attT_A": attT_A, "attT_B": attT_B,
        "xemb_b_cols": c(
            (np.asarray(inputs["gat_bl"], f32) + np.asarray(inputs["gat_bias"], f32))
            .reshape(2, 128).T
        ),
        "glWT_A": c(inputs["gl_W"].T[0:128]),
        "glWT_B": c(inputs["gl_W"].T[128:256]),
        "glb_m": col(inputs["gl_b"][0:1]), "glb_lv": col(inputs["gl_b"][1:2]),
        "dec_WihT": c(inputs["dec_Wih"].T),
        "dec_WhhT": c(inputs["dec_Whh"].T),
        "dec_brz": col(inputs["dec_bih"][0:128] + inputs["dec_bhh"][0:128]),
        "dec_bin": col(inputs["dec_bih"][128:192]),
        "dec_bhn": col(inputs["dec_bhh"][128:192]),
        "fcWT_aug": np.concatenate(
            [c(inputs["fc_W"].T), np.asarray(inputs["fc_b"], f32)[None, :]], axis=0
        ),
    }
    wpack = np.zeros((128, SMALL_F), f32)
    off = 0
    for name, p, f in SMALL_W:
        a = wvals[name].reshape(p, f)
        assert a.shape == (p, f), (name, a.shape)
        wpack[0:p, off : off + f] = a
        off += f
    bigpacks = [np.zeros((128, BIG_F), f32) for _ in range(4)]
    for k in range(4):
        off = 0
        for name, f in BIG_W:
            bigpacks[k][:, off : off + f] = wvals[name][k * 128 : (k + 1) * 128]
            off += f
    weights = {"wpack": wpack}
    for k in range(4):
        weights[f"bigpack{k}"] = bigpacks[k]
    import ml_dtypes
    weights["rpack"] = np.concatenate(
        [wvals["attT_A"], wvals["attT_B"]], axis=1
    ).astype(ml_dtypes.bfloat16)

    esn = np.asarray(inputs["es_now"], f32)
    esl = np.asarray(inputs["es_lag"], f32)
    prior = np.asarray(inputs["prior_adj"], f32)

    in_maps = []
    for cidx in range(NC):
        sl = slice(cidx * BC, (cidx + 1) * BC)
        m = {
            "xt": np.ascontiguousarray(XT[:, :, sl]),
            "eps1T": np.ascontiguousarray(eps1T[:, sl]),
            "eps2T": np.ascontiguousarray(eps2T[:, sl]),
            "tcT": np.ascontiguousarray(tcT[:, sl]),
            "es_now_sh": np.ascontiguousarray(esn[sl]),
            "es_lag_sh": np.ascontiguousarray(esl[sl]),
            "prior_sh": np.ascontiguousarray(prior[sl]),
        }
        m.update(weights)
        in_maps.append(m)
    return in_maps


def run(inputs, trace=False):
    from concourse.bass_utils import run_bass_kernel_spmd

    nc = _get_prog()
    in_maps = _host_prep(inputs)
    res = run_bass_kernel_spmd(
        nc, in_maps, core_ids=list(range(NC)), trace=trace
    )
    rs = res.results

    recon = np.concatenate(
        [np.transpose(r["o_recon"], (1, 0, 2)) for r in rs], axis=0
    )
    mu = np.concatenate([r["o_muT"].T for r in rs], axis=0)
    logvar = np.concatenate([r["o_lvT"].T for r in rs], axis=0)
    adjn = np.concatenate([r["o_adjn"] for r in rs], axis=0)
    adjl = np.concatenate([r["o_adjl"] for r in rs], axis=0)
    idx = np.arange(N)
    adjn[idx, idx] = 1e-8
    adjl[idx, idx] = 1e-8
    mean = np.concatenate([r["o_meanT"][0] for r in rs])[:, None]
    std = np.concatenate([r["o_stdT"][0] for r in rs])[:, None]

    out = (
        recon.astype(np.float32),
        mu.astype(np.float32),
        logvar.astype(np.float32),
        adjn.astype(np.float32),
        adjl.astype(np.float32),
        mean.astype(np.float32),
        std.astype(np.float32),
    )
    if trace:
        return out, res.exec_time_ns
    return out


def kernel(**inputs):
    return run(inputs)
